# revision 14
# baseline (speedup 1.0000x reference)
"""Trainium2 Bass kernel for nn_CausalSelfAttention_10368051052888.

Head-sharded tensor parallel over 8 NeuronCores (2 heads/core).
Feature-major ("transposed") layout on device: activations live as
[feature, seq] so the PE contraction dim is always the partition dim.

Schedule (two dense phases, PE kept busy end to end):
  Phase 1: ONE streamed pass over xT computing q, k AND v for each of
    the 6 seq slices (q/k feature-major via weight-stationary matmuls,
    v seq-major via x-stationary matmuls), plus sum-of-squares partials
    on the PE (ones-matmul). The two chunked ssq AllGathers fire early
    (after slice 1 / slice 5); their recv chains (GpSimd 8-row reduce +
    ACT sqrt + DVE reciprocal) and the first ropes run under the tail
    of the x stream, so attention starts the moment phase 1 ends.
  Phase 2: pure attention pipeline. Per slice: scores/exp/AV for both
    heads, then a deferred "finish" per head (fp32 pacc fold -> bf16 ->
    PE ones-matmul partition reduce -> DVE reciprocal -> GpSimd
    broadcast -> normalize), ONE merged AllGather per slice (both
    heads), and the previous slice's output projection. Denominator
    partition-reduction moved from GpSimd (6.7us each) to a 0.3us PE
    ones-matmul. Remaining ropes (slices 2-5) are slotted between
    attention calls on the otherwise-slack DVE.

Per core c (heads 2c, 2c+1):
  - attention scores in [k, q] orientation, exp without max-subtraction
    (max|s| ~ 6.5), denominators via bf16 group accumulation on DVE
  - merged AllGather of normalized attention outputs (bf16), then
    column-sharded output projection y[:, c*256:(c+1)*256]

Self-contained: hardcodes the problem shapes from the spec.
"""
import numpy as np
import ml_dtypes

import concourse.bass as bass
import concourse.bass_isa as bass_isa
import concourse.mybir as mybir
import concourse.tile as tile
from concourse import bacc
from concourse.bass_utils import run_bass_kernel_spmd

BF = ml_dtypes.bfloat16

N_CORES = 8
S = 2640
D = 2048
H = 16
HD = 128
CACHE = 5280
EPS = 1e-6

HPC = H // N_CORES          # heads per core = 2
MF = HPC * HD               # per-core feature slice = 256
L = CACHE + S               # 7920
KC = D // 128               # 16 contraction chunks
CTILES = (CACHE + 127) // 128   # 42 cache k-tiles (last kt=32)
NTILES = (S + 127) // 128       # 21 new k-tiles / v s-tiles (last 80)
VPAD = CTILES * 128             # 5376 padded cache rows for v
NQ = 512
# query slices: 128-aligned starts; five full 512s plus an 80-wide tail
# (the tail's small-N matmuls are overhead-dominated but cheap in total,
# and a tiny final slice shrinks the end-of-kernel AllGather exposure)
N_SLICES = [(0, 512), (512, 512), (1024, 512), (1536, 512),
            (2048, 512), (2560, 80)]
NJ = len(N_SLICES)
# ssq AllGather chunks: slice 0 alone fires at ~26us so its result
# clears the CC control plane (~35us barrier-relative latency) right
# as phase 1's PE work drains; slices 1-5 fire at phase-1 end and
# their recv hides under slice-0 attention. Keeping this at TWO ops
# matters: every CC op costs ~25us+ of serial control-plane latency,
# and the attention-output AllGathers queue behind the ssq ones.
AR_CHUNKS = [(0, 1), (1, 6)]

SWAP_MASK = [(i ^ 1) for i in range(32)]  # pair swap within 32-partition groups

_prog_cache = {}


def build_program():
    dt = mybir.dt
    f32, bf16 = dt.float32, dt.bfloat16
    nc = bacc.Bacc("TRN2", target_bir_lowering=False, debug=False,
                   num_devices=N_CORES)

    # ---------------- I/O ----------------
    xT = nc.dram_tensor("xT", [D, S], bf16, kind="ExternalInput")
    wq = nc.dram_tensor("wq", [128, KC * MF], bf16, kind="ExternalInput")
    wk = nc.dram_tensor("wk", [128, KC * MF], bf16, kind="ExternalInput")
    wv = nc.dram_tensor("wv", [128, KC * MF], bf16, kind="ExternalInput")
    wo = nc.dram_tensor("wo", [128, KC * MF], bf16, kind="ExternalInput")
    cosT = nc.dram_tensor("cosT", [128, S], bf16, kind="ExternalInput")
    sinT = nc.dram_tensor("sinT", [128, S], bf16, kind="ExternalInput")
    kTc = nc.dram_tensor("kTc", [HPC, 128, CACHE], bf16, kind="ExternalInput")
    vc = nc.dram_tensor("vc", [HPC, 128, VPAD], bf16, kind="ExternalInput")
    masks = nc.dram_tensor("masks", [4, 128, NQ], bf16, kind="ExternalInput")
    yT = nc.dram_tensor("yT", [MF, S], f32, kind="ExternalOutput")

    # chunked ssq reduction buffers: [1, 2*w] packing [q-chunk | k-chunk].
    # Cross-core reduction is an AllGather of partials + a local 8-row
    # GpSimd reduce — much lower latency than a tiny AllReduce.
    ssq_in_d = []
    ssq_out_d = []
    for ci, (j0, j1) in enumerate(AR_CHUNKS):
        w = N_SLICES[j1 - 1][0] + N_SLICES[j1 - 1][1] - N_SLICES[j0][0]
        ssq_in_d.append(nc.dram_tensor(f"ssq_in{ci}", [1, 2 * w], bf16))
        ssq_out_d.append(nc.dram_tensor(f"ssq_out{ci}", [N_CORES, 2 * w],
                                        bf16, addr_space="Shared"))
    # merged per-slice AllGather of both heads' normalized outputs
    ag_in = [nc.dram_tensor(f"ag_in{j}", [HPC, 128, nn], bf16)
             for j, (qb, nn) in enumerate(N_SLICES)]
    ag_out = [nc.dram_tensor(f"ag_out{j}", [N_CORES, HPC, 128, nn], bf16,
                             addr_space="Shared")
              for j, (qb, nn) in enumerate(N_SLICES)]

    RG = [list(range(N_CORES))]
    Exp = mybir.ActivationFunctionType.Exp
    Sqrt = mybir.ActivationFunctionType.Sqrt
    add_op = mybir.AluOpType.add
    mult_op = mybir.AluOpType.mult

    with tile.TileContext(nc) as tc:
        with (
            tc.tile_pool(name="const", bufs=1) as constp,
            tc.tile_pool(name="xs", bufs=5) as xsp,
            tc.tile_pool(name="work", bufs=2) as workp,
            tc.tile_pool(name="ftmp", bufs=3) as ftmp,
            tc.tile_pool(name="attn", bufs=2) as attnp,
            tc.tile_pool(name="ptp", bufs=3) as ptp,
            tc.tile_pool(name="psac", bufs=4, space="PSUM") as psac,
            tc.tile_pool(name="pssc", bufs=2, space="PSUM") as pssc,
        ):
            # ------------ persistent SBUF + prologue DMAs ------------
            w_sb = {}
            for name in ("q", "k", "v", "o"):
                w_sb[name] = constp.tile([128, KC * MF], bf16,
                                         tag=f"w{name}", name=f"w{name}")
            # quarter-granularity weight loads so the first matmuls only
            # wait on 0.25MB each; all constant loads ride the scalar
            # (ACT) queue, which does no compute until phase 2's exps
            QW = KC * MF // 4
            for quarter in range(4):
                sl_ = slice(quarter * QW, (quarter + 1) * QW)
                nc.scalar.dma_start(out=w_sb["q"][:, sl_], in_=wq[:, sl_])
                nc.scalar.dma_start(out=w_sb["k"][:, sl_], in_=wk[:, sl_])
                nc.scalar.dma_start(out=w_sb["v"][:, sl_], in_=wv[:, sl_])
            cos_sb = constp.tile([128, S], bf16, tag="cos")
            sin_sb = constp.tile([128, S], bf16, tag="sin")
            mask_sb = constp.tile([128, 4 * NQ], bf16, tag="masks")
            kT_sb = []
            v_sb = []
            for h in range(HPC):
                kt_t = constp.tile([128, L], bf16, tag=f"kT{h}", name=f"kT{h}")
                kT_sb.append(kt_t)
                v_t = constp.tile([128, VPAD + NTILES * 128], bf16,
                                  tag=f"v{h}", name=f"vsb{h}")
                v_sb.append(v_t)

            def load_bulk_a():
                nc.scalar.dma_start(out=cos_sb[:], in_=cosT[:])
                nc.scalar.dma_start(out=sin_sb[:], in_=sinT[:])
                nc.scalar.dma_start(out=kT_sb[0][:, :CACHE], in_=kTc[0])
                nc.scalar.dma_start(out=v_sb[0][:, :VPAD], in_=vc[0])

            def load_bulk_b():
                nc.scalar.dma_start(out=kT_sb[1][:, :CACHE], in_=kTc[1])
                nc.scalar.dma_start(out=v_sb[1][:, :VPAD], in_=vc[1])
                nc.scalar.dma_start(
                    out=mask_sb[:].rearrange("p (d c) -> p d c", c=NQ),
                    in_=masks[:].rearrange("d p c -> p d c"),
                )
                nc.scalar.dma_start(out=w_sb["o"][:], in_=wo[:])
            # rq_sb doubles as the q staging buffer (rope runs in place);
            # k stages directly into kT_sb[:, CACHE:].
            rq_sb = [constp.tile([128, S], bf16, tag=f"rq{h}", name=f"rq{h}")
                     for h in range(HPC)]
            onescol = constp.tile([128, 1], bf16, tag="onescol")
            nc.vector.memset(onescol[:], 1.0)
            # per-AR-chunk ssq partial tiles
            ssq_c = []
            for ci, (j0, j1) in enumerate(AR_CHUNKS):
                w = (N_SLICES[j1 - 1][0] + N_SLICES[j1 - 1][1]
                     - N_SLICES[j0][0])
                ssq_c.append([
                    constp.tile([1, w], bf16, tag=f"ssq_c{ci}_{ti}",
                                name=f"ssq_c{ci}_{ti}")
                    for ti in range(2)])
            chunk_of = {}
            for ci, (j0, j1) in enumerate(AR_CHUNKS):
                for j in range(j0, j1):
                    chunk_of[j] = ci
            # bf16 rsqrt rows used by rope (q row 0 / k row 1)
            srow16 = [constp.tile([1, S], bf16, tag=f"srow16_{i}",
                                  name=f"srow16_{i}")
                      for i in range(2)]
            eps_col = constp.tile([1, 1], f32, tag="eps")
            nc.vector.memset(eps_col[:], EPS)

            def stage_dest(tname, m, qb, nn):
                if tname == "q":
                    return rq_sb[m][:, qb:qb + nn]
                return kT_sb[m][:, CACHE + qb:CACHE + qb + nn]

            def chunk_cols(ci):
                j0, j1 = AR_CHUNKS[ci]
                c0 = N_SLICES[j0][0]
                w = N_SLICES[j1 - 1][0] + N_SLICES[j1 - 1][1] - c0
                return c0, w, slice(c0, c0 + w)

            def fire_ar(ci):
                c0, w, sl = chunk_cols(ci)
                nc.gpsimd.dma_start(out=ssq_in_d[ci][:, :w],
                                    in_=ssq_c[ci][0][:])
                nc.gpsimd.dma_start(out=ssq_in_d[ci][:, w:],
                                    in_=ssq_c[ci][1][:])
                nc.gpsimd.collective_compute(
                    "AllGather", mybir.AluOpType.bypass, replica_groups=RG,
                    ins=[ssq_in_d[ci][:]], outs=[ssq_out_d[ci][:]])

            def recv_ar(ci):
                c0, w, sl = chunk_cols(ci)
                # gather-readback + local 8-row reduce, in <=1024-col pieces;
                # piece-major over (p0, ti) so the lowest-token srows (which
                # unblock the next rope) land first
                for p0 in range(0, w, 2 * NQ):
                    pw = min(2 * NQ, w - p0)
                    for ti in range(2):
                        parts = workp.tile([N_CORES, 2 * NQ], bf16,
                                           tag="ssqparts", bufs=1,
                                           name="ssq_parts")
                        nc.gpsimd.dma_start(
                            out=parts[:, :pw],
                            in_=ssq_out_d[ci][:, ti * w + p0:ti * w + p0 + pw])
                        nc.gpsimd.partition_all_reduce(
                            parts[:, :pw], parts[:, :pw], channels=N_CORES,
                            reduce_op=bass_isa.ReduceOp.add)
                        # srow = 1/sqrt(ssq/D + eps), then bf16 for rope
                        scr = workp.tile([1, 2 * NQ], f32, tag="sq_scr",
                                         bufs=1, name="sq_scr")
                        nc.scalar.activation(scr[:, :pw],
                                             parts[:1, :pw], Sqrt,
                                             scale=1.0 / D, bias=eps_col[:])
                        nc.vector.reciprocal_approx_fast(out=scr[:, :pw],
                                                         in_=scr[:, :pw])
                        nc.vector.tensor_copy(
                            srow16[ti][:, c0 + p0:c0 + p0 + pw],
                            scr[:, :pw])

            # ---- phase 1: merged q+k+v projection per slice ----
            def proj_slice(jsl):
                qb, nn = N_SLICES[jsl]
                nst = (nn + 127) // 128
                pst = {t: [psac.tile([128, NQ], f32, tag="acc",
                                     name=f"proj_{t}{m}")
                           for m in range(HPC)] for t in ("q", "k")}
                pv = pssc.tile([128, 2 * NQ], f32, tag="scores", name="pv_ps")
                xsg = []
                for g in range(KC // 4):
                    xs = xsp.tile([128, 4 * NQ], bf16, tag="xs", name="xs")
                    xsg.append(xs)
                    # odd groups ride the gpsimd queue: the x stream runs at
                    # 2x queue bandwidth, which keeps the PE fed through the
                    # just-in-time buffer turnover at slice boundaries
                    q_eng = nc.gpsimd if g % 2 == 1 else nc.sync
                    q_eng.dma_start(
                        out=xs[:].rearrange("p (a n) -> p a n", n=NQ)[:, :, :nn],
                        in_=xT[g * 512:(g + 1) * 512, qb:qb + nn]
                            .rearrange("(a p) n -> p a n", p=128))
                    for kcl in range(4):
                        kc = g * 4 + kcl
                        rhs = xs[:, kcl * NQ:kcl * NQ + nn]
                        for t in ("q", "k"):
                            for m in range(HPC):
                                nc.tensor.matmul(
                                    pst[t][m][:, :nn],
                                    w_sb[t][:, kc * MF + m * 128:
                                            kc * MF + (m + 1) * 128],
                                    rhs, start=(kc == 0), stop=(kc == KC - 1))
                # v after the q/k stream: one 16-kc accumulation chain per
                # seq-subtile, sequential per PSUM bank (one pending
                # accumulation group per bank is a hardware constraint)
                for s_ in range(nst):
                    sw = min(128, nn - s_ * 128)
                    for g in range(KC // 4):
                        for kcl in range(4):
                            kc = g * 4 + kcl
                            nc.tensor.matmul(
                                pv[:sw, s_ * MF:(s_ + 1) * MF],
                                xsg[g][:, kcl * NQ + s_ * 128:
                                       kcl * NQ + s_ * 128 + sw],
                                w_sb["v"][:, kc * MF:(kc + 1) * MF],
                                start=(kc == 0), stop=(kc == KC - 1))
                # drain: stage q/k as bf16, squares + ssq ones-matmuls
                sqp = pssc.tile([128, 2 * NQ], f32, tag="scores", name="sqp")
                ci = chunk_of[jsl]
                lo = qb - N_SLICES[AR_CHUNKS[ci][0]][0]
                for ti, t in enumerate(("q", "k")):
                    for m in range(HPC):
                        st = stage_dest(t, m, qb, nn)
                        nc.vector.tensor_copy(st, pst[t][m][:, :nn])
                        q2 = workp.tile([128, NQ], bf16, tag="btmp", name="q2")
                        nc.vector.tensor_tensor(q2[:, :nn], st, st, mult_op)
                        nc.tensor.matmul(sqp[:1, ti * NQ:ti * NQ + nn],
                                         onescol[:], q2[:, :nn],
                                         start=(m == 0), stop=(m == HPC - 1))
                    nc.vector.tensor_copy(ssq_c[ci][ti][:, lo:lo + nn],
                                          sqp[:1, ti * NQ:ti * NQ + nn])
                # v drain into seq-major per-head v_sb
                for s_ in range(nst):
                    sw = min(128, nn - s_ * 128)
                    st_glob = (qb + s_ * 128) // 128
                    for h in range(HPC):
                        nc.vector.tensor_copy(
                            v_sb[h][:sw, VPAD + st_glob * 128:
                                    VPAD + st_glob * 128 + 128],
                            pv[:sw, s_ * MF + h * 128:
                               s_ * MF + (h + 1) * 128])

            # ---------------- rope ----------------
            def rope_j(j):
                qb, nn = N_SLICES[j]
                for m in range(HPC):
                    for ti, tname in enumerate(("q", "k")):
                        st = stage_dest(tname, m, qb, nn)
                        sh = workp.tile([128, NQ], bf16, tag="btmp", name="sh")
                        nc.vector.stream_shuffle(sh[:, :nn], st, SWAP_MASK)
                        a = ftmp.tile([128, NQ], bf16, tag="btmp2",
                                      name="rope_a")
                        nc.vector.tensor_tensor(
                            a[:, :nn], st, cos_sb[:, qb:qb + nn], mult_op)
                        b = ftmp.tile([128, NQ], bf16, tag="btmp2",
                                      name="rope_b")
                        nc.vector.tensor_tensor(
                            b[:, :nn], sh[:, :nn], sin_sb[:, qb:qb + nn],
                            mult_op)
                        nc.vector.tensor_tensor(a[:, :nn], a[:, :nn],
                                                b[:, :nn], add_op)
                        srb = workp.tile([128, NQ], bf16, tag="srowb", name="srb")
                        nc.gpsimd.partition_broadcast(
                            srb[:, :nn], srow16[ti][:, qb:qb + nn])
                        nc.vector.tensor_tensor(st, a[:, :nn], srb[:, :nn],
                                                mult_op)

            scale = float(HD) ** -0.5
            GSZ = 8   # pairs per bf16 partial-sum group (16 k-tiles)

            def yproj_setup(j):
                return [psac.tile([128, NQ], f32, tag="acc", name="py_ps")
                        for _ in range(HPC)]

            def yproj_group(j, py, g):
                qb, nn = N_SLICES[j]
                gt = xsp.tile([128, 4 * NQ], bf16, tag="xs", name="gt")
                nc.sync.dma_start(
                    out=gt[:].rearrange("p (a n) -> p a n", n=NQ)[:, :, :nn],
                    in_=ag_out[j].rearrange("c h p n -> (c h p) n")
                        [g * 512:(g + 1) * 512, :]
                        .rearrange("(a p) n -> p a n", p=128))
                for kcl in range(4):
                    kc = g * 4 + kcl
                    for m in range(HPC):
                        nc.tensor.matmul(
                            py[m][:, :nn],
                            w_sb["o"][:, kc * MF + m * 128:
                                      kc * MF + (m + 1) * 128],
                            gt[:, kcl * NQ:kcl * NQ + nn],
                            start=(kc == 0), stop=(kc == KC - 1))

            def yproj_drain(j, py):
                qb, nn = N_SLICES[j]
                for m in range(HPC):
                    ys = ftmp.tile([128, NQ], f32, tag="f32tmp", name="ys")
                    nc.vector.tensor_copy(ys[:, :nn], py[m][:, :nn])
                    nc.sync.dma_start(
                        out=yT[m * 128:(m + 1) * 128, qb:qb + nn],
                        in_=ys[:, :nn])

            def yproj_thunks(j):
                py = yproj_setup(j)
                th = [lambda g=g: yproj_group(j, py, g)
                      for g in range(KC // 4)]
                th.append(lambda: yproj_drain(j, py))
                return th

            def make_attn(j, h):
                """Build per-pair emission thunks for (j, h) plus a finisher.
                Thunks from both heads are interleaved 1:1 by the caller so
                ACT exp of one head's pair overlaps PE scores of the other's
                — decoupling the near-lockstep exp->AV dependency chain."""
                qb, nn = N_SLICES[j]
                # k-tile list: (col0 in kT_sb, kt, vcol0, mask_off)
                tiles = []
                for ct in range(CTILES):
                    kt = min(128, CACHE - ct * 128)
                    tiles.append((ct * 128, kt, ct * 128, None))
                for t in range(NTILES):
                    kb = t * 128
                    if kb > qb + nn - 1:
                        continue
                    kt = min(128, S - kb)
                    moff = (kb - qb) if (kb + kt - 1) > qb else None
                    tiles.append((CACHE + kb, kt, VPAD + kb, moff))
                # pair up consecutive full tiles to halve per-instruction
                # overheads on ACT/DVE
                pairs = []
                i = 0
                while i < len(tiles):
                    if (i + 1 < len(tiles) and tiles[i][1] == 128
                            and tiles[i + 1][1] == 128):
                        pairs.append((tiles[i], tiles[i + 1]))
                        i += 2
                    else:
                        pairs.append((tiles[i],))
                        i += 1
                out_ps = psac.tile([128, NQ], f32, tag="acc", name="out_ps")
                pacc = attnp.tile([128, 2 * NQ], f32, tag="pacc", bufs=2, name="pacc")
                rq_slice = rq_sb[h][:, qb:qb + nn]
                st = {"nidx": 0, "gacc": None, "gcount": 0,
                      "pacc_init": False}
                nlast = len(tiles) - 1

                def flush():
                    if st["gacc"] is None:
                        return
                    gv = st["gacc"][:].rearrange(
                        "p (a n) -> p a n", n=NQ)[:, :, :nn]
                    pv_ = pacc[:].rearrange(
                        "p (a n) -> p a n", n=NQ)[:, :, :nn]
                    if st["pacc_init"]:
                        nc.vector.tensor_tensor(pv_, pv_, gv, add_op)
                    else:
                        nc.vector.tensor_copy(pv_, gv)
                    st["gacc"] = None
                    st["gcount"] = 0
                    st["pacc_init"] = True

                def emit_pair(pair):
                    full_pair = len(pair) == 2
                    sc = pssc.tile([128, 2 * NQ], f32, tag="scores", name="sc")
                    for half, (c0, kt, vcol, moff) in enumerate(pair):
                        nc.tensor.matmul(
                            sc[:kt, half * NQ:half * NQ + nn],
                            kT_sb[h][:, c0:c0 + kt],
                            rq_slice, start=True, stop=True)
                    kt0 = pair[0][1]
                    # exp of a group's first full pair writes the group
                    # accumulator directly (saves a DVE copy per group)
                    new_group = full_pair and st["gacc"] is None
                    if new_group:
                        st["gacc"] = attnp.tile([128, 2 * NQ], bf16,
                                                tag="gacc", bufs=2,
                                                name="gacc")
                        pt = st["gacc"]
                        st["gcount"] = 1
                    else:
                        pt = ptp.tile([128, 2 * NQ], bf16, tag="pT", name="pt")
                    if full_pair:
                        nc.scalar.activation(
                            pt[:].rearrange("p (a n) -> p a n",
                                            n=NQ)[:, :, :nn],
                            sc[:].rearrange("p (a n) -> p a n",
                                            n=NQ)[:, :, :nn],
                            Exp, scale=scale)
                    else:
                        nc.scalar.activation(pt[:kt0, :nn],
                                             sc[:kt0, :nn], Exp,
                                             scale=scale)
                    for half, (c0, kt, vcol, moff) in enumerate(pair):
                        if moff is not None:
                            mi = moff // 128
                            nc.vector.tensor_tensor(
                                pt[:kt, half * NQ:half * NQ + nn],
                                pt[:kt, half * NQ:half * NQ + nn],
                                mask_sb[:kt, mi * NQ:mi * NQ + nn],
                                mult_op)
                    # denominator accumulation: bf16 groups of GSZ pairs,
                    # folded into fp32 pacc; odd tiles direct
                    if full_pair:
                        if not new_group:
                            nc.vector.tensor_tensor(
                                st["gacc"][:].rearrange(
                                    "p (a n) -> p a n", n=NQ)[:, :, :nn],
                                st["gacc"][:].rearrange(
                                    "p (a n) -> p a n", n=NQ)[:, :, :nn],
                                pt[:].rearrange("p (a n) -> p a n",
                                                n=NQ)[:, :, :nn],
                                add_op)
                            st["gcount"] += 1
                        if st["gcount"] == GSZ:
                            flush()
                    else:
                        flush()
                        if st["pacc_init"]:
                            nc.vector.tensor_tensor(
                                pacc[:kt0, :nn], pacc[:kt0, :nn],
                                pt[:kt0, :nn], add_op)
                        else:
                            nc.vector.tensor_copy(pacc[:kt0, :nn],
                                                  pt[:kt0, :nn])
                            st["pacc_init"] = True
                    for half, (c0, kt, vcol, moff) in enumerate(pair):
                        nc.tensor.matmul(
                            out_ps[:, :nn],
                            v_sb[h][:kt, vcol:vcol + 128],
                            pt[:kt, half * NQ:half * NQ + nn],
                            start=(st["nidx"] == 0),
                            stop=(st["nidx"] == nlast))
                        st["nidx"] += 1

                thunks = [lambda pair=pair: emit_pair(pair)
                          for pair in pairs]

                def finish(dps, dcol):
                    # fold the two pacc halves straight to bf16, reduce over
                    # partitions with a PE ones-matmul (replacing a 6.7us
                    # GpSimd partition reduce), reciprocal, broadcast
                    flush()
                    den16 = attnp.tile([128, NQ], bf16, tag="den16", bufs=2,
                                       name="den16")
                    nc.vector.tensor_tensor(den16[:, :nn], pacc[:, :nn],
                                            pacc[:, NQ:NQ + nn], add_op)
                    nc.tensor.matmul(dps[:1, dcol:dcol + nn], onescol[:],
                                     den16[:, :nn], start=True, stop=True)
                    rec32 = workp.tile([1, NQ], f32, tag="recf", bufs=1,
                                       name="rec32")
                    nc.vector.reciprocal_approx_fast(
                        out=rec32[:, :nn], in_=dps[:1, dcol:dcol + nn])
                    rec = attnp.tile([1, NQ], bf16, tag="rec", bufs=2, name="rec")
                    nc.vector.tensor_copy(rec[:, :nn], rec32[:, :nn])
                    recb = attnp.tile([128, NQ], bf16, tag="recb", bufs=2, name="recb")
                    nc.gpsimd.partition_broadcast(recb[:, :nn],
                                                  rec[:, :nn])
                    onorm = attnp.tile([128, NQ], bf16, tag="onorm", bufs=2,
                                       name="onorm")
                    nc.vector.tensor_tensor(onorm[:, :nn], out_ps[:, :nn],
                                            recb[:, :nn], mult_op)
                    nc.gpsimd.dma_start(out=ag_in[j][h][:, :nn],
                                        in_=onorm[:, :nn])
                return thunks, finish

            def fire_ag(j):
                nc.gpsimd.collective_compute(
                    "AllGather", mybir.AluOpType.bypass, replica_groups=RG,
                    ins=[ag_in[j][:]], outs=[ag_out[j][:]])

            def emit_slice(j, sprinkle=(), yp=None, yp_frac=0.55):
                """Interleave both heads' pair thunks 1:1, with yproj(j-1)
                groups and recv/rope thunks sprinkled at given fractions."""
                ypth = yproj_thunks(yp) if yp is not None else []
                th0, fin0 = make_attn(j, 0)
                th1, fin1 = make_attn(j, 1)
                merged = []
                for a, b in zip(th0, th1):
                    merged.append(a)
                    merged.append(b)
                inserts = []   # (position, thunk), position in merged list
                n = len(merged)
                for fr, th in sprinkle:
                    inserts.append((int(fr * n), th))
                if ypth:
                    # yproj groups from yp_frac onward (the AllGather they
                    # read lands ~25-45us after the previous slice fired it)
                    for gi, th in enumerate(ypth):
                        fr = yp_frac + (0.97 - yp_frac) * gi / (len(ypth) - 1)
                        inserts.append((int(fr * n), th))
                inserts.sort(key=lambda x: x[0])
                ii = 0
                for pos, th in enumerate(merged):
                    while ii < len(inserts) and inserts[ii][0] <= pos:
                        inserts[ii][1]()
                        ii += 1
                    th()
                while ii < len(inserts):
                    inserts[ii][1]()
                    ii += 1
                dps = pssc.tile([128, 2 * NQ], f32, tag="scores",
                                name="den_ps")
                fin0(dps, 0)
                fin1(dps, NQ)
                fire_ag(j)

            # ================= phase 1 =================
            proj_slice(0)
            fire_ar(0)
            load_bulk_a()
            proj_slice(1)
            load_bulk_b()
            proj_slice(2)
            proj_slice(3)
            proj_slice(4)
            proj_slice(5)
            fire_ar(1)
            recv_ar(0)
            rope_j(0)

            # ================= phase 2 =================
            # recv of the big ssq chunk + rope(1)/rope(2) ride late in
            # slice 0's stream (the AllGather lands ~45us after phase 1);
            # remaining ropes sit mid-slice where DVE has slack
            emit_slice(0, sprinkle=[
                (0.85, lambda: recv_ar(1)),
                (0.88, lambda: rope_j(1)),
                (0.94, lambda: rope_j(2)),
            ])
            emit_slice(1, sprinkle=[(0.5, lambda: rope_j(3))],
                       yp=0, yp_frac=0.60)
            emit_slice(2, sprinkle=[(0.5, lambda: rope_j(4))], yp=1)
            emit_slice(3, sprinkle=[(0.5, lambda: rope_j(5))], yp=2)
            emit_slice(4, yp=3)
            emit_slice(5)
            for th in yproj_thunks(4):
                th()
            for th in yproj_thunks(5):
                th()
    nc.compile()
    return nc


def get_program():
    if "nc" not in _prog_cache:
        _prog_cache["nc"] = build_program()
    return _prog_cache["nc"]


def prep_inputs(x, freqs, k_cache, v_cache, Wq, bq, Wk, bk, Wv, bv, Wo, bo,
                gq, gk, current_start):
    """Host-side sharding/layout. Returns per-core in_maps."""
    cs = int(current_start)
    x = np.asarray(x, dtype=np.float32)
    xT = np.ascontiguousarray(x[0].T).astype(BF)           # [D, S]
    freqs = np.asarray(freqs, dtype=np.float32)
    csl = freqs[cs:cs + S, :HD // 2]                       # [S, 64]
    snl = freqs[cs:cs + S, HD // 2:]                       # [S, 64]
    cosT = np.empty((128, S), np.float32)
    sinT = np.empty((128, S), np.float32)
    cosT[0::2] = csl.T
    cosT[1::2] = csl.T
    sinT[0::2] = -snl.T
    sinT[1::2] = snl.T
    cosT = cosT.astype(BF)
    sinT = sinT.astype(BF)
    # spec guarantees zero biases and unit gains; the device program
    # relies on that (cheap to add back via K=1 bias matmuls if needed)
    for b in (bq, bk, bv, bo):
        assert not np.any(np.asarray(b)), "nonzero bias unsupported"
    for g in (gq, gk):
        assert np.all(np.asarray(g) == 1.0), "non-unit gain unsupported"
    # masks: multiplicative {0,1}, mask_d[r, c] = 1 if c >= r + d
    masks = np.zeros((4, 128, NQ), np.float32)
    r = np.arange(128)[:, None]
    c = np.arange(NQ)[None, :]
    for di, d in enumerate((0, 128, 256, 384)):
        masks[di] = (c >= r + d).astype(np.float32)
    masks = masks.astype(BF)

    k_cache = np.asarray(k_cache, np.float32)
    v_cache = np.asarray(v_cache, np.float32)

    def wlayout(W, sl):
        wt = np.ascontiguousarray(np.asarray(W, np.float32)[sl].T)  # [D, MF]
        return np.ascontiguousarray(
            wt.reshape(KC, 128, MF).transpose(1, 0, 2)
            .reshape(128, KC * MF)).astype(BF)

    in_maps = []
    for core in range(N_CORES):
        h0 = core * HPC
        sl = slice(core * MF, (core + 1) * MF)
        kTc = np.ascontiguousarray(
            np.transpose(k_cache[:, h0:h0 + HPC, :], (1, 2, 0))).astype(BF)
        vpad = np.zeros((HPC, 128, VPAD), BF)
        for h in range(HPC):
            vt = np.zeros((VPAD, HD), np.float32)
            vt[:CACHE] = v_cache[:, h0 + h, :]
            vpad[h] = np.ascontiguousarray(
                vt.reshape(CTILES, 128, HD).transpose(1, 0, 2)
                .reshape(128, VPAD)).astype(BF)
        in_maps.append({
            "xT": xT,
            "wq": wlayout(Wq, sl),
            "wk": wlayout(Wk, sl),
            "wv": wlayout(Wv, sl),
            "wo": wlayout(Wo, sl),
            "cosT": cosT,
            "sinT": sinT,
            "kTc": kTc,
            "vc": vpad,
            "masks": masks,
        })
    return in_maps


def assemble_output(results):
    cols = [np.asarray(r["yT"], np.float32).T for r in results]  # [S, MF] each
    return np.ascontiguousarray(np.concatenate(cols, axis=1))[None]


def run(inputs, trace=False):
    nc = get_program()
    in_maps = prep_inputs(**inputs)
    r = run_bass_kernel_spmd(nc, in_maps, core_ids=list(range(N_CORES)),
                             trace=trace)
    return assemble_output(r.results), r


def kernel(**inputs):
    out, _ = run(inputs, trace=False)
    return out


# revision 19
# speedup vs baseline: 1.0061x; 1.0061x over previous
"""Trainium2 Bass kernel for nn_CausalSelfAttention_10368051052888.

Head-sharded tensor parallel over 8 NeuronCores (2 heads/core).
Feature-major ("transposed") layout on device: activations live as
[feature, seq] so the PE contraction dim is always the partition dim.

Schedule (two dense phases, PE kept busy end to end):
  Phase 1: ONE streamed pass over xT computing q, k AND v for each of
    the 6 seq slices (q/k feature-major via weight-stationary matmuls,
    v seq-major via x-stationary matmuls), plus sum-of-squares partials
    on the PE (ones-matmul). The two chunked ssq AllGathers fire early
    (after slice 1 / slice 5); their recv chains (GpSimd 8-row reduce +
    ACT sqrt + DVE reciprocal) and the first ropes run under the tail
    of the x stream, so attention starts the moment phase 1 ends.
  Phase 2: pure attention pipeline. Per slice: scores/exp/AV for both
    heads, then a deferred "finish" per head (fp32 pacc fold -> bf16 ->
    PE ones-matmul partition reduce -> DVE reciprocal -> GpSimd
    broadcast -> normalize), ONE merged AllGather per slice (both
    heads), and the previous slice's output projection. Denominator
    partition-reduction moved from GpSimd (6.7us each) to a 0.3us PE
    ones-matmul. Remaining ropes (slices 2-5) are slotted between
    attention calls on the otherwise-slack DVE.

Per core c (heads 2c, 2c+1):
  - attention scores in [k, q] orientation, exp without max-subtraction
    (max|s| ~ 6.5), denominators via bf16 group accumulation on DVE
  - merged AllGather of normalized attention outputs (bf16), then
    column-sharded output projection y[:, c*256:(c+1)*256]

Self-contained: hardcodes the problem shapes from the spec.
"""
import numpy as np
import ml_dtypes

import concourse.bass as bass
import concourse.bass_isa as bass_isa
import concourse.mybir as mybir
import concourse.tile as tile
from concourse import bacc
from concourse.bass_utils import run_bass_kernel_spmd

BF = ml_dtypes.bfloat16

N_CORES = 8
S = 2640
D = 2048
H = 16
HD = 128
CACHE = 5280
EPS = 1e-6

HPC = H // N_CORES          # heads per core = 2
MF = HPC * HD               # per-core feature slice = 256
L = CACHE + S               # 7920
KC = D // 128               # 16 contraction chunks
CTILES = (CACHE + 127) // 128   # 42 cache k-tiles (last kt=32)
NTILES = (S + 127) // 128       # 21 new k-tiles / v s-tiles (last 80)
VPAD = CTILES * 128             # 5376 padded cache rows for v
NQ = 512
# query slices: 128-aligned starts; five full 512s plus an 80-wide tail
# (the tail's small-N matmuls are overhead-dominated but cheap in total,
# and a tiny final slice shrinks the end-of-kernel AllGather exposure)
N_SLICES = [(0, 512), (512, 512), (1024, 512), (1536, 512),
            (2048, 512), (2560, 80)]
NJ = len(N_SLICES)
# ssq AllGather chunks: slice 0 alone fires at ~26us so its result
# clears the CC control plane (~60us fire->done latency) right as
# phase 1's PE work drains; (1,2) fires after slice 2 so rope(1) is
# ready well before attention reaches slice 1; (3,4,5) fires at
# phase-1 end and its recv hides under slice-0/1 attention.
AR_CHUNKS = [(0, 1), (1, 3), (3, 6)]

SWAP_MASK = [(i ^ 1) for i in range(32)]  # pair swap within 32-partition groups

_prog_cache = {}


def build_program():
    dt = mybir.dt
    f32, bf16 = dt.float32, dt.bfloat16
    nc = bacc.Bacc("TRN2", target_bir_lowering=False, debug=False,
                   num_devices=N_CORES)

    # ---------------- I/O ----------------
    xT = nc.dram_tensor("xT", [D, S], bf16, kind="ExternalInput")
    wq = nc.dram_tensor("wq", [128, KC * MF], bf16, kind="ExternalInput")
    wk = nc.dram_tensor("wk", [128, KC * MF], bf16, kind="ExternalInput")
    wv = nc.dram_tensor("wv", [128, KC * MF], bf16, kind="ExternalInput")
    wo = nc.dram_tensor("wo", [128, KC * MF], bf16, kind="ExternalInput")
    cosT = nc.dram_tensor("cosT", [128, S], bf16, kind="ExternalInput")
    sinT = nc.dram_tensor("sinT", [128, S], bf16, kind="ExternalInput")
    kTc = nc.dram_tensor("kTc", [HPC, 128, CACHE], bf16, kind="ExternalInput")
    vc = nc.dram_tensor("vc", [HPC, 128, VPAD], bf16, kind="ExternalInput")
    masks = nc.dram_tensor("masks", [4, 128, NQ], bf16, kind="ExternalInput")
    yT = nc.dram_tensor("yT", [MF, S], f32, kind="ExternalOutput")

    # chunked ssq reduction buffers: [1, 2*w] packing [q-chunk | k-chunk].
    # Cross-core reduction is an AllGather of partials + a local 8-row
    # GpSimd reduce — much lower latency than a tiny AllReduce.
    ssq_in_d = []
    ssq_out_d = []
    for ci, (j0, j1) in enumerate(AR_CHUNKS):
        w = N_SLICES[j1 - 1][0] + N_SLICES[j1 - 1][1] - N_SLICES[j0][0]
        ssq_in_d.append(nc.dram_tensor(f"ssq_in{ci}", [1, 2 * w], bf16))
        ssq_out_d.append(nc.dram_tensor(f"ssq_out{ci}", [N_CORES, 2 * w],
                                        bf16, addr_space="Shared"))
    # merged per-slice AllGather of both heads' normalized outputs; the
    # ~55-110us fire->done latency means yproj(j) runs ~one slice late,
    # which the list scheduler absorbs into otherwise-idle PE time
    ag_in = [nc.dram_tensor(f"ag_in{j}", [HPC, 128, nn], bf16)
             for j, (qb, nn) in enumerate(N_SLICES)]
    ag_out = [nc.dram_tensor(f"ag_out{j}", [N_CORES, HPC, 128, nn], bf16,
                             addr_space="Shared")
              for j, (qb, nn) in enumerate(N_SLICES)]

    RG = [list(range(N_CORES))]
    Exp = mybir.ActivationFunctionType.Exp
    Sqrt = mybir.ActivationFunctionType.Sqrt
    add_op = mybir.AluOpType.add
    mult_op = mybir.AluOpType.mult

    with tile.TileContext(nc) as tc:
        with (
            tc.tile_pool(name="const", bufs=1) as constp,
            tc.tile_pool(name="xs", bufs=5) as xsp,
            tc.tile_pool(name="work", bufs=2) as workp,
            tc.tile_pool(name="ftmp", bufs=3) as ftmp,
            tc.tile_pool(name="attn", bufs=2) as attnp,
            tc.tile_pool(name="ptp", bufs=3) as ptp,
            tc.tile_pool(name="psac", bufs=4, space="PSUM") as psac,
            tc.tile_pool(name="pssc", bufs=2, space="PSUM") as pssc,
        ):
            # ------------ persistent SBUF + prologue DMAs ------------
            w_sb = {}
            for name in ("q", "k", "v", "o"):
                w_sb[name] = constp.tile([128, KC * MF], bf16,
                                         tag=f"w{name}", name=f"w{name}")
            # quarter-granularity weight loads so the first matmuls only
            # wait on 0.25MB each; all constant loads ride the scalar
            # (ACT) queue, which does no compute until phase 2's exps
            QW = KC * MF // 4
            for quarter in range(4):
                sl_ = slice(quarter * QW, (quarter + 1) * QW)
                nc.scalar.dma_start(out=w_sb["q"][:, sl_], in_=wq[:, sl_])
                nc.scalar.dma_start(out=w_sb["k"][:, sl_], in_=wk[:, sl_])
            for quarter in range(4):
                sl_ = slice(quarter * QW, (quarter + 1) * QW)
                nc.scalar.dma_start(out=w_sb["v"][:, sl_], in_=wv[:, sl_])
            cos_sb = constp.tile([128, S], bf16, tag="cos")
            sin_sb = constp.tile([128, S], bf16, tag="sin")
            mask_sb = constp.tile([128, 4 * NQ], bf16, tag="masks")
            kT_sb = []
            v_sb = []
            for h in range(HPC):
                kt_t = constp.tile([128, L], bf16, tag=f"kT{h}", name=f"kT{h}")
                kT_sb.append(kt_t)
                v_t = constp.tile([128, VPAD + NTILES * 128], bf16,
                                  tag=f"v{h}", name=f"vsb{h}")
                v_sb.append(v_t)

            def load_bulk_a():
                nc.scalar.dma_start(out=cos_sb[:], in_=cosT[:])
                nc.scalar.dma_start(out=sin_sb[:], in_=sinT[:])
                nc.scalar.dma_start(out=kT_sb[0][:, :CACHE], in_=kTc[0])
                nc.scalar.dma_start(out=v_sb[0][:, :VPAD], in_=vc[0])

            def load_bulk_b():
                nc.scalar.dma_start(out=kT_sb[1][:, :CACHE], in_=kTc[1])
                nc.scalar.dma_start(out=v_sb[1][:, :VPAD], in_=vc[1])
                nc.scalar.dma_start(
                    out=mask_sb[:].rearrange("p (d c) -> p d c", c=NQ),
                    in_=masks[:].rearrange("d p c -> p d c"),
                )
                nc.scalar.dma_start(out=w_sb["o"][:], in_=wo[:])
            # rq_sb doubles as the q staging buffer (rope runs in place);
            # k stages directly into kT_sb[:, CACHE:].
            rq_sb = [constp.tile([128, S], bf16, tag=f"rq{h}", name=f"rq{h}")
                     for h in range(HPC)]
            onescol = constp.tile([128, 1], bf16, tag="onescol")
            nc.vector.memset(onescol[:], 1.0)
            # per-AR-chunk ssq partial tiles
            ssq_c = []
            for ci, (j0, j1) in enumerate(AR_CHUNKS):
                w = (N_SLICES[j1 - 1][0] + N_SLICES[j1 - 1][1]
                     - N_SLICES[j0][0])
                ssq_c.append([
                    constp.tile([1, w], bf16, tag=f"ssq_c{ci}_{ti}",
                                name=f"ssq_c{ci}_{ti}")
                    for ti in range(2)])
            chunk_of = {}
            for ci, (j0, j1) in enumerate(AR_CHUNKS):
                for j in range(j0, j1):
                    chunk_of[j] = ci
            # bf16 rsqrt rows used by rope (q row 0 / k row 1)
            srow16 = [constp.tile([1, S], bf16, tag=f"srow16_{i}",
                                  name=f"srow16_{i}")
                      for i in range(2)]
            eps_col = constp.tile([1, 1], f32, tag="eps")
            nc.vector.memset(eps_col[:], EPS)

            def stage_dest(tname, m, qb, nn):
                if tname == "q":
                    return rq_sb[m][:, qb:qb + nn]
                return kT_sb[m][:, CACHE + qb:CACHE + qb + nn]

            def chunk_cols(ci):
                j0, j1 = AR_CHUNKS[ci]
                c0 = N_SLICES[j0][0]
                w = N_SLICES[j1 - 1][0] + N_SLICES[j1 - 1][1] - c0
                return c0, w, slice(c0, c0 + w)

            def fire_ar(ci):
                c0, w, sl = chunk_cols(ci)
                nc.gpsimd.dma_start(out=ssq_in_d[ci][:, :w],
                                    in_=ssq_c[ci][0][:])
                nc.gpsimd.dma_start(out=ssq_in_d[ci][:, w:],
                                    in_=ssq_c[ci][1][:])
                nc.gpsimd.collective_compute(
                    "AllGather", mybir.AluOpType.bypass, replica_groups=RG,
                    ins=[ssq_in_d[ci][:]], outs=[ssq_out_d[ci][:]])

            def recv_ar(ci):
                c0, w, sl = chunk_cols(ci)
                # gather-readback + local 8-row reduce, in <=1024-col pieces;
                # piece-major over (p0, ti) so the lowest-token srows (which
                # unblock the next rope) land first
                for p0 in range(0, w, 2 * NQ):
                    pw = min(2 * NQ, w - p0)
                    for ti in range(2):
                        parts = workp.tile([N_CORES, 2 * NQ], bf16,
                                           tag="ssqparts", bufs=1,
                                           name="ssq_parts")
                        nc.gpsimd.dma_start(
                            out=parts[:, :pw],
                            in_=ssq_out_d[ci][:, ti * w + p0:ti * w + p0 + pw])
                        nc.gpsimd.partition_all_reduce(
                            parts[:, :pw], parts[:, :pw], channels=N_CORES,
                            reduce_op=bass_isa.ReduceOp.add)
                        # srow = 1/sqrt(ssq/D + eps), then bf16 for rope
                        scr = workp.tile([1, 2 * NQ], f32, tag="sq_scr",
                                         bufs=1, name="sq_scr")
                        nc.scalar.activation(scr[:, :pw],
                                             parts[:1, :pw], Sqrt,
                                             scale=1.0 / D, bias=eps_col[:])
                        nc.vector.reciprocal_approx_fast(out=scr[:, :pw],
                                                         in_=scr[:, :pw])
                        nc.vector.tensor_copy(
                            srow16[ti][:, c0 + p0:c0 + p0 + pw],
                            scr[:, :pw])

            # ---- phase 1: merged q+k+v projection per slice ----
            def proj_slice(jsl):
                qb, nn = N_SLICES[jsl]
                nst = (nn + 127) // 128
                pst = {t: [psac.tile([128, NQ], f32, tag="acc",
                                     name=f"proj_{t}{m}")
                           for m in range(HPC)] for t in ("q", "k")}
                pv = pssc.tile([128, 2 * NQ], f32, tag="scores", name="pv_ps")
                xsg = []
                for g in range(KC // 4):
                    xs = xsp.tile([128, 4 * NQ], bf16, tag="xs", name="xs")
                    xsg.append(xs)
                    # odd groups ride a second queue (2x stream bandwidth
                    # through the just-in-time buffer turnover at slice
                    # boundaries); slices 1-2 use the scalar queue because
                    # the gpsimd queue may be blocked on the AR0 trigger's
                    # collective credits until the initial barrier clears
                    alt = nc.scalar if jsl in (1, 2) else nc.gpsimd
                    q_eng = alt if g % 2 == 1 else nc.sync
                    q_eng.dma_start(
                        out=xs[:].rearrange("p (a n) -> p a n", n=NQ)[:, :, :nn],
                        in_=xT[g * 512:(g + 1) * 512, qb:qb + nn]
                            .rearrange("(a p) n -> p a n", p=128))
                    for kcl in range(4):
                        kc = g * 4 + kcl
                        rhs = xs[:, kcl * NQ:kcl * NQ + nn]
                        for t in ("q", "k"):
                            for m in range(HPC):
                                nc.tensor.matmul(
                                    pst[t][m][:, :nn],
                                    w_sb[t][:, kc * MF + m * 128:
                                            kc * MF + (m + 1) * 128],
                                    rhs, start=(kc == 0), stop=(kc == KC - 1))
                # v after the q/k stream: one 16-kc accumulation chain per
                # seq-subtile, sequential per PSUM bank (one pending
                # accumulation group per bank is a hardware constraint)
                for s_ in range(nst):
                    sw = min(128, nn - s_ * 128)
                    for g in range(KC // 4):
                        for kcl in range(4):
                            kc = g * 4 + kcl
                            nc.tensor.matmul(
                                pv[:sw, s_ * MF:(s_ + 1) * MF],
                                xsg[g][:, kcl * NQ + s_ * 128:
                                       kcl * NQ + s_ * 128 + sw],
                                w_sb["v"][:, kc * MF:(kc + 1) * MF],
                                start=(kc == 0), stop=(kc == KC - 1))
                # drain: stage q/k as bf16, squares + ssq ones-matmuls
                sqp = pssc.tile([128, 2 * NQ], f32, tag="scores", name="sqp")
                ci = chunk_of[jsl]
                lo = qb - N_SLICES[AR_CHUNKS[ci][0]][0]
                for ti, t in enumerate(("q", "k")):
                    for m in range(HPC):
                        st = stage_dest(t, m, qb, nn)
                        nc.vector.tensor_copy(st, pst[t][m][:, :nn])
                        q2 = workp.tile([128, NQ], bf16, tag="btmp", name="q2")
                        nc.vector.tensor_tensor(q2[:, :nn], st, st, mult_op)
                        nc.tensor.matmul(sqp[:1, ti * NQ:ti * NQ + nn],
                                         onescol[:], q2[:, :nn],
                                         start=(m == 0), stop=(m == HPC - 1))
                    nc.vector.tensor_copy(ssq_c[ci][ti][:, lo:lo + nn],
                                          sqp[:1, ti * NQ:ti * NQ + nn])
                # v drain into seq-major per-head v_sb
                for s_ in range(nst):
                    sw = min(128, nn - s_ * 128)
                    st_glob = (qb + s_ * 128) // 128
                    for h in range(HPC):
                        nc.vector.tensor_copy(
                            v_sb[h][:sw, VPAD + st_glob * 128:
                                    VPAD + st_glob * 128 + 128],
                            pv[:sw, s_ * MF + h * 128:
                               s_ * MF + (h + 1) * 128])

            # ---------------- rope ----------------
            def rope_j(j):
                qb, nn = N_SLICES[j]
                for m in range(HPC):
                    for ti, tname in enumerate(("q", "k")):
                        st = stage_dest(tname, m, qb, nn)
                        sh = workp.tile([128, NQ], bf16, tag="btmp", name="sh")
                        nc.vector.stream_shuffle(sh[:, :nn], st, SWAP_MASK)
                        a = ftmp.tile([128, NQ], bf16, tag="btmp2",
                                      name="rope_a")
                        nc.vector.tensor_tensor(
                            a[:, :nn], st, cos_sb[:, qb:qb + nn], mult_op)
                        b = ftmp.tile([128, NQ], bf16, tag="btmp2",
                                      name="rope_b")
                        nc.vector.tensor_tensor(
                            b[:, :nn], sh[:, :nn], sin_sb[:, qb:qb + nn],
                            mult_op)
                        nc.vector.tensor_tensor(a[:, :nn], a[:, :nn],
                                                b[:, :nn], add_op)
                        srb = workp.tile([128, NQ], bf16, tag="srowb", name="srb")
                        nc.gpsimd.partition_broadcast(
                            srb[:, :nn], srow16[ti][:, qb:qb + nn])
                        nc.vector.tensor_tensor(st, a[:, :nn], srb[:, :nn],
                                                mult_op)

            scale = float(HD) ** -0.5
            GSZ = 8   # pairs per bf16 partial-sum group (16 k-tiles)

            def yproj_setup(j):
                return [psac.tile([128, NQ], f32, tag="acc", name="py_ps")
                        for _ in range(HPC)]

            def yproj_group(j, py, g):
                qb, nn = N_SLICES[j]
                gt = xsp.tile([128, 4 * NQ], bf16, tag="xs", name="gt")
                nc.sync.dma_start(
                    out=gt[:].rearrange("p (a n) -> p a n", n=NQ)[:, :, :nn],
                    in_=ag_out[j].rearrange("c h p n -> (c h p) n")
                        [g * 512:(g + 1) * 512, :]
                        .rearrange("(a p) n -> p a n", p=128))
                for kcl in range(4):
                    kc = g * 4 + kcl
                    for m in range(HPC):
                        nc.tensor.matmul(
                            py[m][:, :nn],
                            w_sb["o"][:, kc * MF + m * 128:
                                      kc * MF + (m + 1) * 128],
                            gt[:, kcl * NQ:kcl * NQ + nn],
                            start=(kc == 0), stop=(kc == KC - 1))

            def yproj_drain(j, py):
                qb, nn = N_SLICES[j]
                for m in range(HPC):
                    ys = ftmp.tile([128, NQ], f32, tag="f32tmp", name="ys")
                    nc.vector.tensor_copy(ys[:, :nn], py[m][:, :nn])
                    nc.sync.dma_start(
                        out=yT[m * 128:(m + 1) * 128, qb:qb + nn],
                        in_=ys[:, :nn])

            def yproj_thunks(j):
                py = yproj_setup(j)
                th = [lambda g=g: yproj_group(j, py, g)
                      for g in range(KC // 4)]
                th.append(lambda: yproj_drain(j, py))
                return th

            def make_attn(j, h):
                """Build per-pair emission thunks for (j, h) plus a finisher.
                Thunks from both heads are interleaved 1:1 by the caller so
                ACT exp of one head's pair overlaps PE scores of the other's
                — decoupling the near-lockstep exp->AV dependency chain."""
                qb, nn = N_SLICES[j]
                # k-tile list: (col0 in kT_sb, kt, vcol0, mask_off)
                tiles = []
                for ct in range(CTILES):
                    kt = min(128, CACHE - ct * 128)
                    tiles.append((ct * 128, kt, ct * 128, None))
                for t in range(NTILES):
                    kb = t * 128
                    if kb > qb + nn - 1:
                        continue
                    kt = min(128, S - kb)
                    moff = (kb - qb) if (kb + kt - 1) > qb else None
                    tiles.append((CACHE + kb, kt, VPAD + kb, moff))
                # pair up consecutive full tiles to halve per-instruction
                # overheads on ACT/DVE
                pairs = []
                i = 0
                while i < len(tiles):
                    if (i + 1 < len(tiles) and tiles[i][1] == 128
                            and tiles[i + 1][1] == 128):
                        pairs.append((tiles[i], tiles[i + 1]))
                        i += 2
                    else:
                        pairs.append((tiles[i],))
                        i += 1
                out_ps = psac.tile([128, NQ], f32, tag="acc", name="out_ps")
                pacc = attnp.tile([128, 2 * NQ], f32, tag="pacc", bufs=2, name="pacc")
                rq_slice = rq_sb[h][:, qb:qb + nn]
                st = {"nidx": 0, "gacc": None, "gcount": 0,
                      "pacc_init": False}
                nlast = len(tiles) - 1

                def flush():
                    if st["gacc"] is None:
                        return
                    gv = st["gacc"][:].rearrange(
                        "p (a n) -> p a n", n=NQ)[:, :, :nn]
                    pv_ = pacc[:].rearrange(
                        "p (a n) -> p a n", n=NQ)[:, :, :nn]
                    if st["pacc_init"]:
                        nc.vector.tensor_tensor(pv_, pv_, gv, add_op)
                    else:
                        nc.vector.tensor_copy(pv_, gv)
                    st["gacc"] = None
                    st["gcount"] = 0
                    st["pacc_init"] = True

                def emit_pair(pair):
                    full_pair = len(pair) == 2
                    sc = pssc.tile([128, 2 * NQ], f32, tag="scores", name="sc")
                    for half, (c0, kt, vcol, moff) in enumerate(pair):
                        nc.tensor.matmul(
                            sc[:kt, half * NQ:half * NQ + nn],
                            kT_sb[h][:, c0:c0 + kt],
                            rq_slice, start=True, stop=True)
                    kt0 = pair[0][1]
                    # exp of a group's first full pair writes the group
                    # accumulator directly (saves a DVE copy per group)
                    new_group = full_pair and st["gacc"] is None
                    if new_group:
                        st["gacc"] = attnp.tile([128, 2 * NQ], bf16,
                                                tag="gacc", bufs=2,
                                                name="gacc")
                        pt = st["gacc"]
                        st["gcount"] = 1
                    else:
                        pt = ptp.tile([128, 2 * NQ], bf16, tag="pT", name="pt")
                    if full_pair:
                        nc.scalar.activation(
                            pt[:].rearrange("p (a n) -> p a n",
                                            n=NQ)[:, :, :nn],
                            sc[:].rearrange("p (a n) -> p a n",
                                            n=NQ)[:, :, :nn],
                            Exp, scale=scale)
                    else:
                        nc.scalar.activation(pt[:kt0, :nn],
                                             sc[:kt0, :nn], Exp,
                                             scale=scale)
                    for half, (c0, kt, vcol, moff) in enumerate(pair):
                        if moff is not None:
                            mi = moff // 128
                            nc.vector.tensor_tensor(
                                pt[:kt, half * NQ:half * NQ + nn],
                                pt[:kt, half * NQ:half * NQ + nn],
                                mask_sb[:kt, mi * NQ:mi * NQ + nn],
                                mult_op)
                    # denominator accumulation: bf16 groups of GSZ pairs,
                    # folded into fp32 pacc; odd tiles direct
                    if full_pair:
                        if not new_group:
                            nc.vector.tensor_tensor(
                                st["gacc"][:].rearrange(
                                    "p (a n) -> p a n", n=NQ)[:, :, :nn],
                                st["gacc"][:].rearrange(
                                    "p (a n) -> p a n", n=NQ)[:, :, :nn],
                                pt[:].rearrange("p (a n) -> p a n",
                                                n=NQ)[:, :, :nn],
                                add_op)
                            st["gcount"] += 1
                        if st["gcount"] == GSZ:
                            flush()
                    else:
                        flush()
                        if st["pacc_init"]:
                            nc.vector.tensor_tensor(
                                pacc[:kt0, :nn], pacc[:kt0, :nn],
                                pt[:kt0, :nn], add_op)
                        else:
                            nc.vector.tensor_copy(pacc[:kt0, :nn],
                                                  pt[:kt0, :nn])
                            st["pacc_init"] = True
                    for half, (c0, kt, vcol, moff) in enumerate(pair):
                        nc.tensor.matmul(
                            out_ps[:, :nn],
                            v_sb[h][:kt, vcol:vcol + 128],
                            pt[:kt, half * NQ:half * NQ + nn],
                            start=(st["nidx"] == 0),
                            stop=(st["nidx"] == nlast))
                        st["nidx"] += 1

                thunks = [lambda pair=pair: emit_pair(pair)
                          for pair in pairs]

                def finish(dps, dcol):
                    # fold the two pacc halves straight to bf16, reduce over
                    # partitions with a PE ones-matmul (replacing a 6.7us
                    # GpSimd partition reduce), reciprocal, broadcast
                    flush()
                    den16 = attnp.tile([128, NQ], bf16, tag="den16", bufs=2,
                                       name="den16")
                    nc.vector.tensor_tensor(den16[:, :nn], pacc[:, :nn],
                                            pacc[:, NQ:NQ + nn], add_op)
                    nc.tensor.matmul(dps[:1, dcol:dcol + nn], onescol[:],
                                     den16[:, :nn], start=True, stop=True)
                    rec32 = workp.tile([1, NQ], f32, tag="recf", bufs=1,
                                       name="rec32")
                    nc.vector.reciprocal_approx_fast(
                        out=rec32[:, :nn], in_=dps[:1, dcol:dcol + nn])
                    rec = attnp.tile([1, NQ], bf16, tag="rec", bufs=2, name="rec")
                    nc.vector.tensor_copy(rec[:, :nn], rec32[:, :nn])
                    recb = attnp.tile([128, NQ], bf16, tag="recb", bufs=2, name="recb")
                    nc.gpsimd.partition_broadcast(recb[:, :nn],
                                                  rec[:, :nn])
                    onorm = attnp.tile([128, NQ], bf16, tag="onorm", bufs=2,
                                       name="onorm")
                    nc.vector.tensor_tensor(onorm[:, :nn], out_ps[:, :nn],
                                            recb[:, :nn], mult_op)
                    nc.gpsimd.dma_start(out=ag_in[j][h][:, :nn],
                                        in_=onorm[:, :nn])
                return thunks, finish

            def fire_ag(j):
                nc.gpsimd.collective_compute(
                    "AllGather", mybir.AluOpType.bypass, replica_groups=RG,
                    ins=[ag_in[j][:]], outs=[ag_out[j][:]])

            def emit_slice(j, sprinkle=(), yp=None, yp_frac=0.55):
                """Interleave both heads' pair thunks 1:1, with yproj(j-1)
                groups and recv/rope thunks sprinkled at given fractions."""
                ypth = yproj_thunks(yp) if yp is not None else []
                th0, fin0 = make_attn(j, 0)
                th1, fin1 = make_attn(j, 1)
                merged = []
                for a, b in zip(th0, th1):
                    merged.append(a)
                    merged.append(b)
                inserts = []   # (position, thunk), position in merged list
                n = len(merged)
                for fr, th in sprinkle:
                    inserts.append((int(fr * n), th))
                if ypth:
                    # yproj groups from yp_frac onward (the AllGather they
                    # read lands ~25-45us after the previous slice fired it)
                    for gi, th in enumerate(ypth):
                        fr = yp_frac + (0.97 - yp_frac) * gi / (len(ypth) - 1)
                        inserts.append((int(fr * n), th))
                inserts.sort(key=lambda x: x[0])
                ii = 0
                for pos, th in enumerate(merged):
                    while ii < len(inserts) and inserts[ii][0] <= pos:
                        inserts[ii][1]()
                        ii += 1
                    th()
                while ii < len(inserts):
                    inserts[ii][1]()
                    ii += 1
                dps = pssc.tile([128, 2 * NQ], f32, tag="scores",
                                name="den_ps")
                fin0(dps, 0)
                fin1(dps, NQ)
                fire_ag(j)

            # ================= phase 1 =================
            proj_slice(0)
            fire_ar(0)
            proj_slice(1)
            proj_slice(2)
            load_bulk_a()
            proj_slice(3)
            proj_slice(4)
            fire_ar(1)
            load_bulk_b()
            proj_slice(5)
            fire_ar(2)
            recv_ar(0)
            rope_j(0)

            # ================= phase 2 =================
            # ssq chunk 1 (slices 1-2) lands mid-slice-0, chunk 2 lands
            # mid-slice-1; ropes follow their recvs at points where the
            # DVE has slack
            emit_slice(0, sprinkle=[
                (0.55, lambda: recv_ar(1)),
                (0.62, lambda: rope_j(1)),
                (0.70, lambda: rope_j(2)),
            ])
            emit_slice(1, sprinkle=[
                (0.45, lambda: recv_ar(2)),
                (0.55, lambda: rope_j(3)),
                (0.80, lambda: rope_j(4)),
            ], yp=0, yp_frac=0.60)
            emit_slice(2, sprinkle=[(0.3, lambda: rope_j(5))], yp=1)
            emit_slice(3, yp=2)
            emit_slice(4, yp=3)
            emit_slice(5)
            for th in yproj_thunks(4):
                th()
            for th in yproj_thunks(5):
                th()
    nc.compile()
    return nc


def get_program():
    if "nc" not in _prog_cache:
        _prog_cache["nc"] = build_program()
    return _prog_cache["nc"]


def prep_inputs(x, freqs, k_cache, v_cache, Wq, bq, Wk, bk, Wv, bv, Wo, bo,
                gq, gk, current_start):
    """Host-side sharding/layout. Returns per-core in_maps."""
    cs = int(current_start)
    x = np.asarray(x, dtype=np.float32)
    xT = np.ascontiguousarray(x[0].T).astype(BF)           # [D, S]
    freqs = np.asarray(freqs, dtype=np.float32)
    csl = freqs[cs:cs + S, :HD // 2]                       # [S, 64]
    snl = freqs[cs:cs + S, HD // 2:]                       # [S, 64]
    cosT = np.empty((128, S), np.float32)
    sinT = np.empty((128, S), np.float32)
    cosT[0::2] = csl.T
    cosT[1::2] = csl.T
    sinT[0::2] = -snl.T
    sinT[1::2] = snl.T
    cosT = cosT.astype(BF)
    sinT = sinT.astype(BF)
    # spec guarantees zero biases and unit gains; the device program
    # relies on that (cheap to add back via K=1 bias matmuls if needed)
    for b in (bq, bk, bv, bo):
        assert not np.any(np.asarray(b)), "nonzero bias unsupported"
    for g in (gq, gk):
        assert np.all(np.asarray(g) == 1.0), "non-unit gain unsupported"
    # masks: multiplicative {0,1}, mask_d[r, c] = 1 if c >= r + d
    masks = np.zeros((4, 128, NQ), np.float32)
    r = np.arange(128)[:, None]
    c = np.arange(NQ)[None, :]
    for di, d in enumerate((0, 128, 256, 384)):
        masks[di] = (c >= r + d).astype(np.float32)
    masks = masks.astype(BF)

    k_cache = np.asarray(k_cache, np.float32)
    v_cache = np.asarray(v_cache, np.float32)

    def wlayout(W, sl):
        wt = np.ascontiguousarray(np.asarray(W, np.float32)[sl].T)  # [D, MF]
        return np.ascontiguousarray(
            wt.reshape(KC, 128, MF).transpose(1, 0, 2)
            .reshape(128, KC * MF)).astype(BF)

    in_maps = []
    for core in range(N_CORES):
        h0 = core * HPC
        sl = slice(core * MF, (core + 1) * MF)
        kTc = np.ascontiguousarray(
            np.transpose(k_cache[:, h0:h0 + HPC, :], (1, 2, 0))).astype(BF)
        vpad = np.zeros((HPC, 128, VPAD), BF)
        for h in range(HPC):
            vt = np.zeros((VPAD, HD), np.float32)
            vt[:CACHE] = v_cache[:, h0 + h, :]
            vpad[h] = np.ascontiguousarray(
                vt.reshape(CTILES, 128, HD).transpose(1, 0, 2)
                .reshape(128, VPAD)).astype(BF)
        in_maps.append({
            "xT": xT,
            "wq": wlayout(Wq, sl),
            "wk": wlayout(Wk, sl),
            "wv": wlayout(Wv, sl),
            "wo": wlayout(Wo, sl),
            "cosT": cosT,
            "sinT": sinT,
            "kTc": kTc,
            "vc": vpad,
            "masks": masks,
        })
    return in_maps


def assemble_output(results):
    cols = [np.asarray(r["yT"], np.float32).T for r in results]  # [S, MF] each
    return np.ascontiguousarray(np.concatenate(cols, axis=1))[None]


def run(inputs, trace=False):
    nc = get_program()
    in_maps = prep_inputs(**inputs)
    r = run_bass_kernel_spmd(nc, in_maps, core_ids=list(range(N_CORES)),
                             trace=trace)
    return assemble_output(r.results), r


def kernel(**inputs):
    out, _ = run(inputs, trace=False)
    return out
